# revision 25
# baseline (speedup 1.0000x reference)
"""Category-specific linear (MoE-routing style) Trainium2 Bass kernel.

Computes out[n] = x[n] @ W[cat_ids[n]] + b[cat_ids[n]] for
x: [N, M, D_IN] f32, cat_ids: [N] int64, W: [C, D_IN, D_H] f32, b: [C, D_H] f32.

Strategy (8-core SPMD, full inputs in / full output out):
  Host: stable-sort samples by category, split into 8 equal shards of
  N/8 samples (perfect load balance).  Within a shard, each category is a
  contiguous run; runs are padded to whole 128-row tiles (8 samples) so the
  device program is fully static.  x rows are pre-transposed on the host
  (fp32 has no DMA-transpose path on TRN2) into [2, 128, NT*128] so the
  contraction dim lands on SBUF partitions.  Each core also gets a small
  deduplicated weight table (its <=KMAX distinct categories) and a per-tile
  weight-slot index.
  Device: the weight table lives in SBUF; for each 128-row tile the weight
  slot index is loaded into a PE register (values_load) and the matmul's
  moving operand is selected with a dynamic slice - zero weight duplication
  in HBM traffic, no dynamic control flow.  Two accumulating matmuls per
  tile (contraction 256 = 2x128), PSUM -> SBUF copy, batched stores.
"""

import os
import sys

import numpy as np

for _p in ("/opt/trn_rl_repo",):
    if os.path.isdir(_p) and _p not in sys.path:
        sys.path.insert(0, _p)

import concourse.bass as bass  # noqa: E402
import concourse.mybir as mybir  # noqa: E402
import concourse.tile as tile  # noqa: E402
from concourse import bacc  # noqa: E402
from concourse.bass import ds  # noqa: E402
from concourse.bass_utils import run_bass_kernel_spmd  # noqa: E402

NCORES = 8
P = 128  # SBUF partitions / rows per tile
D_IN = 256  # contraction dim (2 chunks of 128)
D_H = 256  # output dim
ROWS_PER_SAMPLE = 16
SPT = P // ROWS_PER_SAMPLE  # samples per tile = 8
TB = 8  # tile-count quantum (NT is padded to a multiple of this)
TBI = 16  # tiles per index-register load
OB = 4  # tiles per psum group / DVE copy
OS = 8  # tiles per out-store DMA

# filled by kernel() for test harness introspection
last_results = None


def _pack(x, cat_ids, W):
    """Host-side routing: sort, shard, pad, transpose, dedup weights.

    Returns (in_maps, scatter_info, NT, KMAX).
    scatter_info[k] = (sample_ids_per_padded_slot [NT*SPT] int64, valid mask)
    """
    N, M, Din = x.shape
    assert M == ROWS_PER_SAMPLE and Din == D_IN
    assert N % NCORES == 0
    S = N // NCORES

    cat = np.asarray(cat_ids).astype(np.int64).ravel()
    order = np.argsort(cat, kind="stable")
    cats_sorted = cat[order]

    # global category runs over the sorted sample list
    bounds = np.flatnonzero(np.diff(cats_sorted)) + 1
    seg_starts = np.concatenate([[0], bounds])
    seg_ends = np.concatenate([bounds, [N]])
    segments = [
        (int(cats_sorted[s]), int(s), int(e))
        for s, e in zip(seg_starts, seg_ends)
    ]

    def pack(T):
        """Greedy-pack category runs into cores of <= T tiles each.

        A run cut mid-category always cuts at an SPT-sample multiple, so
        cuts cost no padding; only each core-local run tail pads to a tile.
        Returns (padded_ids, tile_cats) per core or None if > NCORES cores.
        """
        cores = []
        cur_ids, cur_tcats, used = [], [], 0
        rem = list(segments)
        i = 0

        def close():
            nonlocal cur_ids, cur_tcats, used
            cores.append((cur_ids, cur_tcats))
            cur_ids, cur_tcats, used = [], [], 0

        while i < len(rem):
            c, s, e = rem[i]
            n = e - s
            tiles_need = (n + SPT - 1) // SPT
            avail = T - used
            if avail >= tiles_need:
                npad = (-n) % SPT
                cur_ids.append(order[s:e])
                if npad:
                    cur_ids.append(np.full(npad, -1, np.int64))
                cur_tcats.extend([c] * tiles_need)
                used += tiles_need
                i += 1
            elif avail >= 1:
                take = avail * SPT  # n > take since tiles_need > avail
                cur_ids.append(order[s : s + take])
                cur_tcats.extend([c] * avail)
                used = T
                rem[i] = (c, s + take, e)
                close()
            else:
                close()
            if len(cores) > NCORES:
                return None
        if cur_tcats:
            close()
        if len(cores) > NCORES:
            return None
        while len(cores) < NCORES:
            cores.append(([], []))
        return cores

    lo, hi = (N // NCORES) // SPT, ((N // NCORES) // SPT) * 2 + 16
    while lo < hi:
        mid = (lo + hi) // 2
        if pack(mid) is not None:
            hi = mid
        else:
            lo = mid + 1
    NT = ((lo + 3) // 4) * 4  # multiple of OB
    cores = pack(NT)
    assert cores is not None

    # pad every core to NT tiles
    padded_ids = []
    tile_cats = []
    for k in range(NCORES):
        ids_parts, tcats = cores[k]
        n_have = len(tcats)
        extra = NT - n_have
        if extra:
            fill_cat = tcats[0] if tcats else 0
            tcats = tcats + [fill_cat] * extra
            ids_parts = ids_parts + [np.full(extra * SPT, -1, np.int64)]
        padded_ids.append(
            np.concatenate(ids_parts)
            if ids_parts
            else np.full(NT * SPT, -1, np.int64)
        )
        tile_cats.append(tcats)

    # per-core weight dedup
    uniq_list = []
    for k in range(NCORES):
        seen = dict()
        for c in tile_cats[k]:
            if c not in seen:
                seen[c] = len(seen)
        uniq_list.append(seen)
    KMAX = max(len(u) for u in uniq_list)

    np_in = _np_in_dtype()
    in_maps = []
    scatter = []
    for k in range(NCORES):
        ids = padded_ids[k]
        valid = ids >= 0
        # gather + zero-pad x rows: [NT*SPT, M, Din]
        Xr = np.zeros((NT * SPT, M, Din), np.float32)
        Xr[valid] = x[ids[valid]]
        # transpose to [Din, NT*P] then chunk the contraction dim
        xT = np.ascontiguousarray(
            Xr.reshape(NT * P, Din).T.astype(np_in)
        ).reshape(2, P, NT * P)

        seen = uniq_list[k]
        w_ids = list(seen.keys())
        w_ids += [w_ids[0]] * (KMAX - len(w_ids))
        Wp = W[np.asarray(w_ids, np.int64)]  # [KMAX, Din, D_H]
        Wl = np.ascontiguousarray(
            Wp.reshape(KMAX, 2, P, D_H).transpose(2, 1, 0, 3).astype(np_in)
        )  # [P, 2, KMAX, D_H]

        widx = np.asarray([seen[c] for c in tile_cats[k]], np.int32)[None, :]

        in_maps.append({"xT": xT, "Wl": Wl, "widx": widx})
        scatter.append((ids, valid))

    return in_maps, scatter, NT, KMAX


def _dt_mode():
    return os.environ.get("CSL_DT_MODE", "f16")


def _out_mode():
    return os.environ.get("CSL_OUT_DT", "f16")


def _np_in_dtype():
    import ml_dtypes

    return {
        "f16": np.float16,
        "bf16": ml_dtypes.bfloat16,
        "f32r": np.float32,
        "f32": np.float32,
    }[_dt_mode()]


def _mm_dt():
    return {
        "f16": mybir.dt.float16,
        "bf16": mybir.dt.bfloat16,
        "f32r": mybir.dt.float32r,
        "f32": mybir.dt.float32,
    }[_dt_mode()]


def _build(NT, KMAX):
    """Build the SPMD device program for NT tiles and KMAX weight slots."""
    mm_dt = _mm_dt()
    out_dt = mybir.dt.float32 if _out_mode() == "f32" else mybir.dt.float16
    f32 = mybir.dt.float32
    i32 = mybir.dt.int32
    static_idx = os.environ.get("CSL_STATIC", "0") == "1"

    nc = bacc.Bacc(
        "TRN2",
        target_bir_lowering=False,
        debug=False,
        enable_asserts=False,
        num_devices=NCORES,
    )
    NTR = NT * P
    GX = 16  # tiles per x-load DMA group
    xT_d = nc.dram_tensor("xT", [2, P, NTR], mm_dt, kind="ExternalInput").ap()
    W_d = nc.dram_tensor("Wl", [P, 2, KMAX, D_H], mm_dt, kind="ExternalInput").ap()
    wi_d = nc.dram_tensor("widx", [1, NT], i32, kind="ExternalInput").ap()
    # partition-major output layout: fully contiguous per-partition stores;
    # the host untransposes when scattering back
    out_d = nc.dram_tensor("out", [P, NT, D_H], out_dt, kind="ExternalOutput").ap()

    with tile.TileContext(nc) as tc:
        with (
            tc.tile_pool(name="wpool", bufs=1) as wpool,
            tc.tile_pool(name="xpool", bufs=3) as xpool,
            tc.tile_pool(name="opool", bufs=3) as opool,
            tc.tile_pool(name="psum", bufs=4, space="PSUM") as psum_pool,
        ):
            # widx first (tiny, unblocks index loads); W on the Scalar ring
            # so it issues in parallel with the Sync-ring x loads; the ic=0
            # half lands first so tile 0's first matmul can start sooner
            wi_sb = wpool.tile([1, NT], i32)
            nc.sync.dma_start(wi_sb[:], wi_d)
            W_sb = wpool.tile([P, 2, KMAX, D_H], mm_dt)
            nc.scalar.dma_start(W_sb[:, 0], W_d[:, 0])
            nc.scalar.dma_start(W_sb[:, 1], W_d[:, 1])

            for g0 in range(0, NT, GX):
                gx = min(GX, NT - g0)
                # loads on the Sync HWDGE ring; stores go on the Scalar ring
                # so a store waiting on DVE never blocks a prefetch load
                xt = xpool.tile([P, 2, GX * P], mm_dt)
                if g0 == 0:
                    # split the first group so the first tiles arrive early
                    h = gx // 2
                    nc.sync.dma_start(xt[:, 0, : h * P], xT_d[0, :, : h * P])
                    nc.sync.dma_start(xt[:, 1, : h * P], xT_d[1, :, : h * P])
                    nc.sync.dma_start(
                        xt[:, 0, h * P : gx * P], xT_d[0, :, h * P : gx * P]
                    )
                    nc.sync.dma_start(
                        xt[:, 1, h * P : gx * P], xT_d[1, :, h * P : gx * P]
                    )
                else:
                    nc.sync.dma_start(
                        xt[:, 0, : gx * P], xT_d[0, :, g0 * P : (g0 + gx) * P]
                    )
                    nc.sync.dma_start(
                        xt[:, 1, : gx * P], xT_d[1, :, g0 * P : (g0 + gx) * P]
                    )
                for i0 in range(0, gx, TBI):
                    ti = min(TBI, gx - i0)
                    if static_idx:
                        vals = (0,) * ti  # debug: no dynamic indexing
                    else:
                        # one TENSOR_LOAD for ti per-tile weight slots
                        _, vals = nc.values_load_multi_w_load_instructions(
                            wi_sb[0:1, g0 + i0 : g0 + i0 + ti],
                            engines=(mybir.EngineType.PE,),
                            min_val=0,
                            max_val=KMAX - 1,
                            skip_runtime_bounds_check=True,
                        )
                    for s0 in range(0, ti, OS):
                        os_ = min(OS, ti - s0)
                        ot = opool.tile([P, OS, D_H], out_dt)
                        for o0 in range(s0, s0 + os_, OB):
                            ob_ = min(OB, s0 + os_ - o0)
                            ps = psum_pool.tile([P, OB, D_H], f32)
                            for j in range(ob_):
                                tt = i0 + o0 + j  # tile within group
                                widx = vals[o0 + j]
                                nc.tensor.matmul(
                                    ps[:, j, :],
                                    xt[:, 0, tt * P : (tt + 1) * P],
                                    W_sb[:, 0, ds(widx, 1), :],
                                    start=True,
                                    stop=False,
                                )
                                nc.tensor.matmul(
                                    ps[:, j, :],
                                    xt[:, 1, tt * P : (tt + 1) * P],
                                    W_sb[:, 1, ds(widx, 1), :],
                                    start=False,
                                    stop=True,
                                )
                            nc.vector.tensor_copy(
                                ot[:, o0 - s0 : o0 - s0 + ob_], ps[:, :ob_]
                            )
                        t_abs = g0 + i0 + s0
                        nc.scalar.dma_start(
                            out_d[:, t_abs : t_abs + os_, :], ot[:, :os_]
                        )

    nc.compile()
    return nc


def kernel(x=None, cat_ids=None, W=None, b=None, **_unused):
    global last_results
    x = np.asarray(x, np.float32)
    W = np.asarray(W, np.float32)
    N, M, _ = x.shape

    in_maps, scatter, NT, KMAX = _pack(x, cat_ids, W)

    nc = _build(NT, KMAX)

    trace = os.environ.get("CSL_TRACE", "0") == "1"
    kwargs = {}
    if trace:
        kwargs["trace"] = True
        tc_env = os.environ.get("CSL_TRACE_CORES", "")
        if tc_env:
            kwargs["trace_cores"] = [int(c) for c in tc_env.split(",")]
        else:
            kwargs["trace_cores"] = list(range(NCORES))
    res = run_bass_kernel_spmd(
        nc, in_maps, core_ids=list(range(NCORES)), **kwargs
    )
    last_results = res

    out = np.empty((N, M, D_H), np.float32)
    for k in range(NCORES):
        ids, valid = scatter[k]
        # device layout [P, NT, D_H] -> row-major [NT*P, D_H]
        ok = res.results[k]["out"].astype(np.float32, copy=False)
        ok = ok.transpose(1, 0, 2).reshape(NT * SPT, ROWS_PER_SAMPLE, D_H)
        out[ids[valid]] = ok[valid]

    if b is not None:
        b = np.asarray(b, np.float32)
        if np.any(b):
            cat = np.asarray(cat_ids).astype(np.int64).ravel()
            out += b[cat][:, None, :]

    return out


# revision 27
# speedup vs baseline: 1.0549x; 1.0549x over previous
"""Category-specific linear (MoE-routing style) Trainium2 Bass kernel.

Computes out[n] = x[n] @ W[cat_ids[n]] + b[cat_ids[n]] for
x: [N, M, D_IN] f32, cat_ids: [N] int64, W: [C, D_IN, D_H] f32, b: [C, D_H] f32.

Strategy (8-core SPMD, full inputs in / full output out):
  Host: stable-sort samples by category, split into 8 equal shards of
  N/8 samples (perfect load balance).  Within a shard, each category is a
  contiguous run; runs are padded to whole 128-row tiles (8 samples) so the
  device program is fully static.  x rows are pre-transposed on the host
  (fp32 has no DMA-transpose path on TRN2) into [2, 128, NT*128] so the
  contraction dim lands on SBUF partitions.  Each core also gets a small
  deduplicated weight table (its <=KMAX distinct categories) and a per-tile
  weight-slot index.
  Device: the weight table lives in SBUF; for each 128-row tile the weight
  slot index is loaded into a PE register (values_load) and the matmul's
  moving operand is selected with a dynamic slice - zero weight duplication
  in HBM traffic, no dynamic control flow.  Two accumulating matmuls per
  tile (contraction 256 = 2x128), PSUM -> SBUF copy, batched stores.
"""

import os
import sys

import numpy as np

for _p in ("/opt/trn_rl_repo",):
    if os.path.isdir(_p) and _p not in sys.path:
        sys.path.insert(0, _p)

import concourse.bass as bass  # noqa: E402
import concourse.mybir as mybir  # noqa: E402
import concourse.tile as tile  # noqa: E402
from concourse import bacc  # noqa: E402
from concourse.bass import ds  # noqa: E402
from concourse.bass_utils import run_bass_kernel_spmd  # noqa: E402

NCORES = 8
P = 128  # SBUF partitions / rows per tile
D_IN = 256  # contraction dim (2 chunks of 128)
D_H = 256  # output dim
ROWS_PER_SAMPLE = 16
SPT = P // ROWS_PER_SAMPLE  # samples per tile = 8
TB = 8  # tile-count quantum (NT is padded to a multiple of this)
TBI = 16  # tiles per index-register load
OB = 4  # tiles per psum group / DVE copy
OS = 8  # tiles per out-store DMA

# filled by kernel() for test harness introspection
last_results = None


def _pack(x, cat_ids, W):
    """Host-side routing: sort, shard, pad, transpose, dedup weights.

    Returns (in_maps, scatter_info, NT, KMAX).
    scatter_info[k] = (sample_ids_per_padded_slot [NT*SPT] int64, valid mask)
    """
    N, M, Din = x.shape
    assert M == ROWS_PER_SAMPLE and Din == D_IN
    assert N % NCORES == 0
    S = N // NCORES

    cat = np.asarray(cat_ids).astype(np.int64).ravel()
    order = np.argsort(cat, kind="stable")
    cats_sorted = cat[order]

    # global category runs over the sorted sample list
    bounds = np.flatnonzero(np.diff(cats_sorted)) + 1
    seg_starts = np.concatenate([[0], bounds])
    seg_ends = np.concatenate([bounds, [N]])
    segments = [
        (int(cats_sorted[s]), int(s), int(e))
        for s, e in zip(seg_starts, seg_ends)
    ]

    def pack(T):
        """Greedy-pack category runs into cores of <= T tiles each.

        A run cut mid-category always cuts at an SPT-sample multiple, so
        cuts cost no padding; only each core-local run tail pads to a tile.
        Returns (padded_ids, tile_cats) per core or None if > NCORES cores.
        """
        cores = []
        cur_ids, cur_tcats, used = [], [], 0
        rem = list(segments)
        i = 0

        def close():
            nonlocal cur_ids, cur_tcats, used
            cores.append((cur_ids, cur_tcats))
            cur_ids, cur_tcats, used = [], [], 0

        while i < len(rem):
            c, s, e = rem[i]
            n = e - s
            tiles_need = (n + SPT - 1) // SPT
            avail = T - used
            if avail >= tiles_need:
                npad = (-n) % SPT
                cur_ids.append(order[s:e])
                if npad:
                    cur_ids.append(np.full(npad, -1, np.int64))
                cur_tcats.extend([c] * tiles_need)
                used += tiles_need
                i += 1
            elif avail >= 1:
                take = avail * SPT  # n > take since tiles_need > avail
                cur_ids.append(order[s : s + take])
                cur_tcats.extend([c] * avail)
                used = T
                rem[i] = (c, s + take, e)
                close()
            else:
                close()
            if len(cores) > NCORES:
                return None
        if cur_tcats:
            close()
        if len(cores) > NCORES:
            return None
        while len(cores) < NCORES:
            cores.append(([], []))
        return cores

    lo, hi = (N // NCORES) // SPT, ((N // NCORES) // SPT) * 2 + 16
    while lo < hi:
        mid = (lo + hi) // 2
        if pack(mid) is not None:
            hi = mid
        else:
            lo = mid + 1
    NT = ((lo + 3) // 4) * 4  # multiple of OB
    cores = pack(NT)
    assert cores is not None

    # pad every core to NT tiles
    padded_ids = []
    tile_cats = []
    for k in range(NCORES):
        ids_parts, tcats = cores[k]
        n_have = len(tcats)
        extra = NT - n_have
        if extra:
            fill_cat = tcats[0] if tcats else 0
            tcats = tcats + [fill_cat] * extra
            ids_parts = ids_parts + [np.full(extra * SPT, -1, np.int64)]
        padded_ids.append(
            np.concatenate(ids_parts)
            if ids_parts
            else np.full(NT * SPT, -1, np.int64)
        )
        tile_cats.append(tcats)

    # per-core weight dedup
    uniq_list = []
    for k in range(NCORES):
        seen = dict()
        for c in tile_cats[k]:
            if c not in seen:
                seen[c] = len(seen)
        uniq_list.append(seen)
    KMAX = max(len(u) for u in uniq_list)

    np_in = _np_in_dtype()
    in_maps = []
    scatter = []
    for k in range(NCORES):
        ids = padded_ids[k]
        valid = ids >= 0
        # gather + zero-pad x rows: [NT*SPT, M, Din]
        Xr = np.zeros((NT * SPT, M, Din), np.float32)
        Xr[valid] = x[ids[valid]]
        # transpose to [Din, NT*P] then chunk the contraction dim
        xT = np.ascontiguousarray(
            Xr.reshape(NT * P, Din).T.astype(np_in)
        ).reshape(2, P, NT * P)

        seen = uniq_list[k]
        w_ids = list(seen.keys())
        w_ids += [w_ids[0]] * (KMAX - len(w_ids))
        Wp = W[np.asarray(w_ids, np.int64)]  # [KMAX, Din, D_H]
        Wl = np.ascontiguousarray(
            Wp.reshape(KMAX, 2, P, D_H).transpose(2, 1, 0, 3).astype(np_in)
        )  # [P, 2, KMAX, D_H]

        widx = np.asarray([seen[c] for c in tile_cats[k]], np.int32)[None, :]

        in_maps.append({"xT": xT, "Wl": Wl, "widx": widx})
        scatter.append((ids, valid))

    return in_maps, scatter, NT, KMAX


def _dt_mode():
    return os.environ.get("CSL_DT_MODE", "f16")


def _out_mode():
    return os.environ.get("CSL_OUT_DT", "f16")


def _np_in_dtype():
    import ml_dtypes

    return {
        "f16": np.float16,
        "bf16": ml_dtypes.bfloat16,
        "f32r": np.float32,
        "f32": np.float32,
    }[_dt_mode()]


def _mm_dt():
    return {
        "f16": mybir.dt.float16,
        "bf16": mybir.dt.bfloat16,
        "f32r": mybir.dt.float32r,
        "f32": mybir.dt.float32,
    }[_dt_mode()]


def _build(NT, KMAX):
    """Build the SPMD device program for NT tiles and KMAX weight slots."""
    mm_dt = _mm_dt()
    out_dt = mybir.dt.float32 if _out_mode() == "f32" else mybir.dt.float16
    f32 = mybir.dt.float32
    i32 = mybir.dt.int32
    static_idx = os.environ.get("CSL_STATIC", "0") == "1"

    nc = bacc.Bacc(
        "TRN2",
        target_bir_lowering=False,
        debug=False,
        enable_asserts=False,
        num_devices=NCORES,
    )
    NTR = NT * P
    GX = 16  # tiles per x-load DMA group
    xT_d = nc.dram_tensor("xT", [2, P, NTR], mm_dt, kind="ExternalInput").ap()
    W_d = nc.dram_tensor("Wl", [P, 2, KMAX, D_H], mm_dt, kind="ExternalInput").ap()
    wi_d = nc.dram_tensor("widx", [1, NT], i32, kind="ExternalInput").ap()
    # partition-major output layout: fully contiguous per-partition stores;
    # the host untransposes when scattering back
    out_d = nc.dram_tensor("out", [P, NT, D_H], out_dt, kind="ExternalOutput").ap()

    with tile.TileContext(nc) as tc:
        with (
            tc.tile_pool(name="wpool", bufs=1) as wpool,
            tc.tile_pool(name="xpool", bufs=3) as xpool,
            tc.tile_pool(name="opool", bufs=3) as opool,
            tc.tile_pool(name="psum", bufs=4, space="PSUM") as psum_pool,
        ):
            # widx first (tiny, unblocks index loads); W on the Scalar ring
            # so it issues in parallel with the Sync-ring x loads; the ic=0
            # half lands first so tile 0's first matmul can start sooner
            wi_sb = wpool.tile([1, NT], i32)
            nc.sync.dma_start(wi_sb[:], wi_d)
            W_sb = wpool.tile([P, 2, KMAX, D_H], mm_dt)
            nc.scalar.dma_start(W_sb[:, 0], W_d[:, 0])
            nc.scalar.dma_start(W_sb[:, 1], W_d[:, 1])

            for g0 in range(0, NT, GX):
                gx = min(GX, NT - g0)
                # loads on the Sync HWDGE ring; stores go on the Scalar ring
                # so a store waiting on DVE never blocks a prefetch load
                xt = xpool.tile([P, 2, GX * P], mm_dt)
                if g0 == 0:
                    # split the first group so the first tiles arrive early
                    h = gx // 2
                    nc.sync.dma_start(xt[:, 0, : h * P], xT_d[0, :, : h * P])
                    nc.sync.dma_start(xt[:, 1, : h * P], xT_d[1, :, : h * P])
                    nc.sync.dma_start(
                        xt[:, 0, h * P : gx * P], xT_d[0, :, h * P : gx * P]
                    )
                    nc.sync.dma_start(
                        xt[:, 1, h * P : gx * P], xT_d[1, :, h * P : gx * P]
                    )
                else:
                    nc.sync.dma_start(
                        xt[:, 0, : gx * P], xT_d[0, :, g0 * P : (g0 + gx) * P]
                    )
                    nc.sync.dma_start(
                        xt[:, 1, : gx * P], xT_d[1, :, g0 * P : (g0 + gx) * P]
                    )
                for i0 in range(0, gx, TBI):
                    ti = min(TBI, gx - i0)
                    if static_idx:
                        vals = (0,) * ti  # debug: no dynamic indexing
                    else:
                        # one TENSOR_LOAD for ti per-tile weight slots
                        _, vals = nc.values_load_multi_w_load_instructions(
                            wi_sb[0:1, g0 + i0 : g0 + i0 + ti],
                            engines=(mybir.EngineType.PE,),
                            min_val=0,
                            max_val=KMAX - 1,
                            skip_runtime_bounds_check=True,
                        )
                    for s0 in range(0, ti, OS):
                        os_ = min(OS, ti - s0)
                        ot = opool.tile([P, OS, D_H], out_dt)
                        for o0 in range(s0, s0 + os_, OB):
                            ob_ = min(OB, s0 + os_ - o0)
                            ps = psum_pool.tile([P, OB, D_H], f32)
                            for j in range(ob_):
                                tt = i0 + o0 + j  # tile within group
                                widx = vals[o0 + j]
                                nc.tensor.matmul(
                                    ps[:, j, :],
                                    xt[:, 0, tt * P : (tt + 1) * P],
                                    W_sb[:, 0, ds(widx, 1), :],
                                    start=True,
                                    stop=False,
                                )
                                nc.tensor.matmul(
                                    ps[:, j, :],
                                    xt[:, 1, tt * P : (tt + 1) * P],
                                    W_sb[:, 1, ds(widx, 1), :],
                                    start=False,
                                    stop=True,
                                )
                            nc.vector.tensor_copy(
                                ot[:, o0 - s0 : o0 - s0 + ob_], ps[:, :ob_]
                            )
                        t_abs = g0 + i0 + s0
                        nc.scalar.dma_start(
                            out_d[:, t_abs : t_abs + os_, :], ot[:, :os_]
                        )

    nc.compile()
    return nc


def kernel(x=None, cat_ids=None, W=None, b=None, **_unused):
    global last_results
    x = np.asarray(x, np.float32)
    W = np.asarray(W, np.float32)
    N, M, _ = x.shape

    in_maps, scatter, NT, KMAX = _pack(x, cat_ids, W)

    nc = _build(NT, KMAX)

    trace = os.environ.get("CSL_TRACE", "0") == "1"
    kwargs = {}
    if trace:
        kwargs["trace"] = True
        tc_env = os.environ.get("CSL_TRACE_CORES", "")
        if tc_env:
            kwargs["trace_cores"] = [int(c) for c in tc_env.split(",")]
        else:
            kwargs["trace_cores"] = list(range(NCORES))
    res = run_bass_kernel_spmd(
        nc, in_maps, core_ids=list(range(NCORES)), **kwargs
    )
    last_results = res

    out = np.empty((N, M, D_H), np.float32)
    for k in range(NCORES):
        ids, valid = scatter[k]
        # device layout [P, NT, D_H] -> row-major [NT*P, D_H]
        ok = res.results[k]["out"].astype(np.float32, copy=False)
        ok = ok.transpose(1, 0, 2).reshape(NT * SPT, ROWS_PER_SAMPLE, D_H)
        out[ids[valid]] = ok[valid]

    if b is not None:
        b = np.asarray(b, np.float32)
        if np.any(b):
            cat = np.asarray(cat_ids).astype(np.int64).ravel()
            out += b[cat][:, None, :]

    return out
